# revision 18
# baseline (speedup 1.0000x reference)
"""LocallyConnected2d v6: TensorEngine banded-matmul formulation.

out[b,i,j] = sum_{kh,kw} x_pad[b, 2i+kh, 2j+kw] * w[i,j,kh,kw] + bias[i,j]

Sharding: output rows split across 8 cores (14 rows each) so weights shard
too. Per core the computation is expressed as a sum of matmuls that all
accumulate into one PSUM region:

  - contraction dim  pi = (slab row m in an 8-row group g, col cl in a
    16-col window w): 128 rows.  Stationary operand = x transposed
    host-side to [pi, b] tiles (LDWEIGHTS cost scales with columns=64).
  - moving operand   = banded weight matrix Wband[pi, (j, i)] built
    host-side: Wband[(m,cl),(j,i)] = w[i, j, (8g+m)-2i, (16w+cl)-2j]
    where valid, else 0.  Streams from SBUF; f enumerates (j outer,
    i inner).
  - PSUM out [b=64 partitions, f = j*16 + i_local] fp32, 1792 values =
    4 banks; bank boundaries align with j in {32,64,96}.  Bias is
    injected by 4 initial start=True matmuls (ones-column stationary,
    bias vector moving) which also clear has_written for each bank.
  - DMA/PE overlap: weights+x arrive in per-row-group transfers ordered
    g0..g4; group-g matmuls start as soon as their two transfers land.
  - epilogue: per-bank ScalarE copies PSUM->SBUF (fp32->fp16) as each
    bank's accumulation completes, then the output DMA in two halves.
"""

import sys

sys.path.insert(0, "/opt/trn_rl_repo")

import numpy as np

import concourse.bass as bass
import concourse.bacc as bacc
import concourse.mybir as mybir
from concourse.tile import TileContext
from concourse.bass_utils import run_bass_kernel_spmd

B = 64
H = W = 224
KH = KW = 7
PH = PW = 3
NKH = NKW = 112
NCORES = 8
RPC = NKH // NCORES           # 14 output rows per core
SLAB = 2 * (RPC - 1) + KH     # 33 slab rows per core
CPAD = 240                    # padded slab width (15 windows of 16)
NG = 5                        # row groups of 8 (last has 1 real row)
NWIN = 15                     # column windows of 16
IPAD = 16                     # i padded 14 -> 16 so 512 | 32*IPAD
FOUT = NKW * IPAD             # 1792 psum f32 values = 4 banks

F16 = mybir.dt.float16
F32 = mybir.dt.float32


def _irange(g):
    return max(0, 4 * g - 3), min(13, 4 * g + 3)      # inclusive


def _jrange(w):
    return max(0, 8 * w - 3), min(NKW, 8 * w + 8)     # exclusive hi


def _chunks():
    """(g, w, ilo, ic, jlo, jc, foff); foff resets per row group."""
    out = []
    for g in range(NG):
        ilo, ihi = _irange(g)
        ic = ihi - ilo + 1
        foff = 0
        for w in range(NWIN):
            jlo, jhi = _jrange(w)
            jc = jhi - jlo
            out.append((g, w, ilo, ic, jlo, jc, foff))
            foff += ic * jc
    return out


CHUNKS = _chunks()
SUMJC = sum(_jrange(w)[1] - _jrange(w)[0] for w in range(NWIN))   # 154
ICNT = [_irange(g)[1] - _irange(g)[0] + 1 for g in range(NG)]     # 4,7,7,5,1
FG = [ic * SUMJC for ic in ICNT]   # wband f-size per row group


def _ap(base, dims):
    return bass.AP(tensor=base.tensor, offset=base.offset,
                   ap=[base.ap[0]] + dims)


def _build_nc(n_iters=1, device_loop=0):
    nc = bacc.Bacc("TRN2", target_bir_lowering=False, debug=False,
                   num_devices=NCORES)

    xw_d = [nc.dram_tensor(f"xw{g}",
                           [16 if g == 4 else 128, FG[g] + NWIN * B],
                           F16, kind="ExternalInput") for g in range(NG)]
    bvon_d = nc.dram_tensor("bvon", [1, FOUT + B], F16, kind="ExternalInput")
    o_d = nc.dram_tensor("o", [B, FOUT], F16, kind="ExternalOutput")

    # matmul piece list: (chunk_idx, j0, j1) with j-splits at bank edges
    pieces = []
    for ci, (g, w, ilo, ic, jlo, jc, foff) in enumerate(CHUNKS):
        edges = [jlo] + [e for e in (32, 64, 96) if jlo < e < jlo + jc] \
            + [jlo + jc]
        for a, b in zip(edges[:-1], edges[1:]):
            pieces.append((ci, a, b))
    last_in_bank = {}
    for pi_, (ci, a, b) in enumerate(pieces):
        last_in_bank[a // 32] = pi_

    from contextlib import nullcontext
    with TileContext(nc) as tc:
        with tc.tile_pool(name="pool", bufs=2) as pool, \
                tc.tile_pool(name="ppool", bufs=2, space="PSUM") as ppool, \
                (tc.For_i(0, device_loop, 1) if device_loop
                 else nullcontext()):
            for it in range(n_iters):
                bvon = pool.tile([1, FOUT + B], F16, tag="bvon")
                xw = [pool.tile([16 if g == 4 else 128, FG[g] + NWIN * B],
                                F16, tag=f"xw{g}", name=f"xw{g}")
                      for g in range(NG)]
                ps = ppool.tile([B, FOUT], F32, tag="ps")
                osb = pool.tile([B, FOUT], F16, tag="osb")

                nc.sync.dma_start(out=bvon[:, :], in_=bvon_d.ap())
                for g in range(NG):
                    nc.sync.dma_start(out=xw[g][:, :], in_=xw_d[g].ap())

                # bias matmuls: start=True clears each bank and seeds bias
                for k in range(4):
                    f0 = 512 * k
                    n = min(512, FOUT - f0)
                    nc.tensor.matmul(
                        out=ps[:, f0:f0 + n],
                        lhsT=bvon[:, FOUT:FOUT + B],
                        rhs=bvon[:, f0:f0 + n], start=True, stop=False)

                # accumulation matmuls, row-group major
                for pi_, (ci, ja, jb) in enumerate(pieces):
                    g, w, ilo, ic, jlo, jc, foff = CHUNKS[ci]
                    nj = jb - ja
                    lhs = xw[g][:, FG[g] + w * B:FG[g] + (w + 1) * B]
                    rhsbase = xw[g][:, 0:1]
                    rhs = bass.AP(
                        tensor=rhsbase.tensor,
                        offset=rhsbase.offset + foff + (ja - jlo) * ic,
                        ap=[rhsbase.ap[0], [1, nj * ic]])
                    outb = ps[:, 0:1]
                    outap = bass.AP(
                        tensor=outb.tensor,
                        offset=outb.offset + ja * IPAD + ilo,
                        ap=[outb.ap[0], [IPAD, nj], [1, ic]])
                    nc.tensor.matmul(
                        out=outap, lhsT=lhs, rhs=rhs, start=False,
                        stop=(last_in_bank[ja // 32] == pi_))

                # per-bank epilogue copies (split across ACT and DVE),
                # then output DMA in two halves
                for k in range(4):
                    f0 = 512 * k
                    n = min(512, FOUT - f0)
                    if k % 2 == 0:
                        nc.scalar.copy(out=osb[:, f0:f0 + n],
                                       in_=ps[:, f0:f0 + n])
                    else:
                        nc.vector.tensor_copy(out=osb[:, f0:f0 + n],
                                              in_=ps[:, f0:f0 + n])
                half = 1024
                nc.sync.dma_start(out=o_d.ap()[:, 0:half],
                                    in_=osb[:, 0:half])
                nc.sync.dma_start(out=o_d.ap()[:, half:FOUT],
                                    in_=osb[:, half:FOUT])

    nc.compile()
    return nc


def _shard_inputs(x, weights, bias):
    x = np.asarray(x, dtype=np.float32)
    weights = np.asarray(weights, dtype=np.float32)
    bias = np.asarray(bias, dtype=np.float32)

    x_pad = np.zeros((B, H + 2 * PH, CPAD), dtype=np.float32)
    x_pad[:, PH:PH + H, PW:PW + W] = x

    in_maps = []
    for c in range(NCORES):
        slab = np.zeros((B, 40, CPAD), dtype=np.float32)
        slab[:, :SLAB] = x_pad[:, 28 * c:28 * c + SLAB, :]
        # xa[p=(m,cl), (g,w), b] = slab[b, 8g+m, 16w+cl]  for g<4
        sl = slab[:, :32, :].reshape(B, 4, 8, NWIN, 16)
        xa = sl.transpose(2, 4, 1, 3, 0).reshape(128, 60 * B)
        x4 = slab[:, 32, :].reshape(B, NWIN, 16).transpose(2, 1, 0) \
            .reshape(16, NWIN * B)

        wc = weights[RPC * c:RPC * (c + 1)]      # [14, 112, 7, 7]
        blocks = {g: [] for g in range(NG)}
        p = np.arange(128)
        m, cl = p // 16, p % 16
        for (g, w, ilo, ic, jlo, jc, foff) in CHUNKS:
            rows = 16 if g == 4 else 128
            r = 8 * g + m[:rows]                            # [rows]
            i = ilo + np.arange(ic)                         # [ic]
            j = jlo + np.arange(jc)                         # [jc]
            kh = r[:, None] - 2 * i[None, :]                # [rows, ic]
            kw = (16 * w + cl[:rows])[:, None] - 2 * j[None, :]
            valid = ((kh >= 0) & (kh < KH))[:, None, :] \
                & ((kw >= 0) & (kw < KW))[:, :, None] \
                & (r < SLAB)[:, None, None]
            vals = wc[i[None, None, :], j[None, :, None],
                      np.clip(kh, 0, KH - 1)[:, None, :],
                      np.clip(kw, 0, KW - 1)[:, :, None]]
            blocks[g].append(
                np.where(valid, vals, 0.0).reshape(rows, jc * ic))
        bvon = np.zeros((NKW, IPAD), dtype=np.float32)
        bvon[:, :RPC] = bias[RPC * c:RPC * (c + 1)].T
        bvon = np.concatenate([bvon.reshape(FOUT),
                               np.ones(B, dtype=np.float32)])
        m_ = {"bvon": bvon.reshape(1, FOUT + B).astype(np.float16)}
        for g in range(NG):
            xg = x4 if g == 4 else xa[:, g * NWIN * B:(g + 1) * NWIN * B]
            m_[f"xw{g}"] = np.concatenate(
                blocks[g] + [xg], axis=1).astype(np.float16)
        in_maps.append(m_)
    return in_maps


def _unshard_output(results):
    outs = []
    for r in results:
        oc = np.asarray(r["o"], dtype=np.float32).reshape(B, NKW, IPAD)
        outs.append(oc[:, :, :RPC].transpose(0, 2, 1))   # [B, 14, 112]
    return np.concatenate(outs, axis=1)                  # [B, 112, 112]


def make_runner(nc, in_maps, require_finite=True):
    """Build a cached jitted runner for nc; returns (run, out_names).
    run() re-executes the NEFF without rebuilding the jit wrapper, so
    repeated calls measure dispatch+exec only."""
    import jax
    from jax.sharding import Mesh, PartitionSpec
    from jax.experimental.shard_map import shard_map
    import concourse.mybir as mybir
    from concourse.bass2jax import (_bass_exec_p, install_neuronx_cc_hook,
                                    partition_id_tensor)

    install_neuronx_cc_hook()
    n_cores = len(in_maps)
    partition_name = (nc.partition_id_tensor.name
                      if nc.partition_id_tensor else None)
    in_names, out_names, out_avals, zero_outs = [], [], [], []
    for alloc in nc.m.functions[0].allocations:
        if not isinstance(alloc, mybir.MemoryLocationSet):
            continue
        name = alloc.memorylocations[0].name
        if alloc.kind == "ExternalInput":
            if name != partition_name:
                in_names.append(name)
        elif alloc.kind == "ExternalOutput":
            shape = tuple(alloc.tensor_shape)
            dtype = mybir.dt.np(alloc.dtype)
            out_names.append(name)
            out_avals.append(jax.core.ShapedArray(shape, dtype))
            zero_outs.append(np.zeros(shape, dtype))
    n_params = len(in_names)
    all_in_names = list(in_names) + list(out_names)
    if partition_name is not None:
        all_in_names.append(partition_name)

    def _body(*args):
        operands = list(args)
        if partition_name is not None:
            operands.append(partition_id_tensor())
        return tuple(_bass_exec_p.bind(
            *operands, out_avals=tuple(out_avals),
            in_names=tuple(all_in_names), out_names=tuple(out_names),
            lowering_input_output_aliases=(),
            sim_require_finite=require_finite,
            sim_require_nnan=require_finite, nc=nc))

    devices = jax.devices()[:n_cores]
    mesh = Mesh(np.asarray(devices), ("core",))
    n_outs = len(out_names)
    sharded = jax.jit(
        shard_map(_body, mesh=mesh,
                  in_specs=(PartitionSpec("core"),) * (n_params + n_outs),
                  out_specs=(PartitionSpec("core"),) * n_outs,
                  check_rep=False),
        donate_argnums=tuple(range(n_params, n_params + n_outs)),
        keep_unused=True)

    concat_in = [np.concatenate([np.asarray(in_maps[c][nm])
                                 for c in range(n_cores)], axis=0)
                 for nm in in_names]
    concat_in = [jax.device_put(a) for a in concat_in]

    def run(queue=1):
        outs = None
        for _ in range(queue):
            zeros = [np.zeros((n_cores * z.shape[0], *z.shape[1:]), z.dtype)
                     for z in zero_outs]
            outs = sharded(*concat_in, *zeros)
        jax.block_until_ready(outs)
        return outs

    def unpack(outs):
        return [{nm: np.asarray(outs[i]).reshape(n_cores, *out_avals[i].shape)[c]
                 for i, nm in enumerate(out_names)} for c in range(n_cores)]

    return run, unpack


_NC_CACHE = None


def _get_nc():
    global _NC_CACHE
    if _NC_CACHE is None:
        _NC_CACHE = _build_nc()
    return _NC_CACHE


def kernel(x, weights, bias):
    nc = _get_nc()
    in_maps = _shard_inputs(x, weights, bias)
    res = run_bass_kernel_spmd(nc, in_maps, core_ids=list(range(NCORES)))
    return _unshard_output(res.results)


def benchmark(x, weights, bias, n_big=2000, reps=5):
    """Serial per-iteration time via an on-device For_i loop."""
    import time

    in_maps = _shard_inputs(x, weights, bias)
    L1, L2 = n_big // 4, n_big
    ncA = _build_nc(1, device_loop=L1)
    runA, _ = make_runner(ncA, in_maps)
    ncB = _build_nc(1, device_loop=L2)
    runB, unpackB = make_runner(ncB, in_maps)
    runA(); outsB = runB()
    tA, tB = [], []
    for _ in range(reps):
        t0 = time.perf_counter(); runA(); tA.append(time.perf_counter() - t0)
        t0 = time.perf_counter(); runB(); tB.append(time.perf_counter() - t0)
    times = {L1: min(tA), L2: min(tB)}
    per_iter_ns = (times[L2] - times[L1]) / (L2 - L1) * 1e9
    return per_iter_ns, times, _unshard_output(unpackB(outsB))
